# revision 14
# baseline (speedup 1.0000x reference)
"""Distributed attention kernel for 8 TRN2 NeuronCores.

Problem: cross-attention (q from target, k/v from reference) with
B=2, N=M=2048, C=1024, H=16 heads, hd=64, followed by an output
projection with bias.

Sharding (data + head parallel):
  core c in 0..7 owns heads {2c, 2c+1} for BOTH batches. It computes
  K^T/Q^T/V for its heads and attention (softmax over keys), producing
  UNNORMALIZED x_local^T [128ch, 2048m] per batch plus per-head softmax
  denominators. One AllToAll per (batch, m-half) redistributes
  [130, 128] blocks (128 channels + 2 denominator rows) so core c owns
  output rows [c*256, (c+1)*256) with ALL 1024 channels; core c then
  normalizes (one reciprocal + broadcast-multiply per half) and applies
  the full Wproj ([1024,1024], replicated) + bias to its row-block.
  Four small collectives instead of two big ones keep the tail short:
  each proj half overlaps the next collective and keeps the PE's HAM
  clock-gate warm.

Schedule: only K-half + first Q-quarter are computed before the
softmax exp stream (the ScalarE roofline of this kernel, ~147us)
starts; the rest of K/Q/V and ALL of batch-1's QKV are emitted as
"fillers" between attention kc-pairs so the PE does them in the gaps
while ACT streams exps back-to-back.

Queue discipline (3 DMA-trigger queues: sync, scalar, gpsimd):
  - scalar carries only pre-attention input loads, then exps: nothing
    pool- or collective-gated may sit in front of an exp.
  - sync carries input chunks + per-m-tile staging into the a2a input.
  - gpsimd carries input chunks, the collective triggers, and all
    proj-side loads (which wait on the collectives) so a slow AllToAll
    can never head-of-line-block the attention/staging path.
  - proj's dsb tiles are allocated from the attention-path "dt" pool
    tag so the pool WAR pins the reciprocal chains BEHIND the
    attention dt casts in the Tile schedule (the scheduler otherwise
    interleaves them optimistically and a late collective then
    head-of-line-blocks the vector queue).

Matmuls run in bf16 (f32 PSUM accumulation); softmax denominators come
free as a ones-column appended to V.
"""

import functools

import numpy as np

B = 2
N = 2048  # reference rows (keys)
M = 2048  # target rows (queries)
C = 1024
H = 16
HD = 64
NCORES = 8
HPC = 2  # heads per core
CHPC = HPC * HD  # 128 channels per core
CHP2 = CHPC + 2  # + 2 denominator rows in the a2a payload
MBLK = M // NCORES  # 256 output rows owned per core (per batch)
MT = 512  # attention m-tile
KC = N // 128  # 16 key chunks
CC = C // 128  # 8 contraction chunks
NMT = M // MT  # 4 m-tiles per batch
HS = MBLK // 2  # 128-col half-slot (per collective)


@functools.lru_cache(maxsize=1)
def _build():
    import concourse.bacc as bacc
    import concourse.mybir as mybir
    import concourse.tile as tile

    fp32 = mybir.dt.float32
    bf16 = mybir.dt.bfloat16
    AF = mybir.ActivationFunctionType

    nc = bacc.Bacc("TRN2", target_bir_lowering=False, debug=False, num_devices=NCORES)

    xrefT = nc.dram_tensor("xrefT", [B, C, N], bf16, kind="ExternalInput")
    xtgtT = nc.dram_tensor("xtgtT", [B, C, M], bf16, kind="ExternalInput")
    wq = nc.dram_tensor("wq", [C, CHPC], bf16, kind="ExternalInput")
    wk = nc.dram_tensor("wk", [C, CHPC], bf16, kind="ExternalInput")
    wv = nc.dram_tensor("wv", [C, CHPC], bf16, kind="ExternalInput")
    wproj = nc.dram_tensor("wproj", [C, C], bf16, kind="ExternalInput")
    bproj = nc.dram_tensor("bproj", [C], fp32, kind="ExternalInput")
    out = nc.dram_tensor("out", [B, C, MBLK], fp32, kind="ExternalOutput")

    with tile.TileContext(nc) as tc:
        with (
            tc.tile_pool(name="wpool", bufs=1) as wpool,
            tc.tile_pool(name="xr", bufs=10) as xrpool,
            tc.tile_pool(name="xt", bufs=10) as xtpool,
            tc.tile_pool(name="kqv", bufs=1) as kqv,
            tc.tile_pool(name="epool", bufs=6) as epool,
            tc.tile_pool(name="spool", bufs=2) as spool,
            tc.tile_pool(name="ppool", bufs=2) as ppool,
            tc.tile_pool(name="psA", bufs=3, space="PSUM") as psA,
            tc.tile_pool(name="psO", bufs=2, space="PSUM") as psO,
            tc.tile_pool(name="dram", bufs=1, space="DRAM") as dpool,
        ):
            # ---- weight loads (emitted first; DMA engines run ahead) ----
            wq_sb = wpool.tile([128, CC, CHPC], bf16)
            wk_sb = wpool.tile([128, CC, CHPC], bf16)
            wv_sb = wpool.tile([128, CC, CHPC], bf16)
            for cc in range(CC):
                nc.sync.dma_start(wk_sb[:, cc, :], wk[cc * 128:(cc + 1) * 128, :])
                nc.gpsimd.dma_start(wv_sb[:, cc, :], wv[cc * 128:(cc + 1) * 128, :])
                nc.scalar.dma_start(wq_sb[:, cc, :], wq[cc * 128:(cc + 1) * 128, :])

            kT = [kqv.tile([128, N], bf16, tag=f"kT{b}", name=f"kT{b}") for b in range(B)]
            qT = [kqv.tile([128, M], bf16, tag=f"qT{b}", name=f"qT{b}") for b in range(B)]
            vA = [
                kqv.tile([128, KC, HPC, HD + 1], bf16, tag=f"vA{b}", name=f"vA{b}")
                for b in range(B)
            ]
            oU = [kqv.tile([128, M], bf16, tag=f"oU{b}", name=f"oU{b}") for b in range(B)]
            for b in range(B):
                nc.vector.memset(vA[b][:, :, :, HD:HD + 1], 1.0)

            # one collective per (batch, m-half): slot j rows 0-127 are the
            # sender's channels, rows 128-129 its two heads' denominators
            a2a_in = [
                [
                    dpool.tile([NCORES, CHP2, HS], bf16, name=f"a2a_in{b}_{hf}")
                    for hf in range(2)
                ]
                for b in range(B)
            ]
            a2a_out = [
                [
                    dpool.tile([NCORES, CHP2, HS], bf16, name=f"a2a_out{b}_{hf}")
                    for hf in range(2)
                ]
                for b in range(B)
            ]
            # bounce for reciprocal'd denominators, [hh, src-core, m]: a
            # [1, 8, m] slice broadcasts across each 64-partition head group
            rdram = dpool.tile([B, 2, 2, NCORES, HS], bf16, name="rdram")

            xch = {}  # (tensor, b, cc) -> sbuf chunk tile

            def load_chunk(which, b, cc, half=None, eng=None):
                src = xrefT if which == "r" else xtgtT
                pool = xrpool if which == "r" else xtpool
                key = (which, b, cc)
                if key in xch:
                    t = xch[key]
                else:
                    t = pool.tile([128, N], bf16, tag="x", name=f"x{which}{b}_{cc}")
                    xch[key] = t
                if half is None:
                    eng.dma_start(t[:], src[b, cc * 128:(cc + 1) * 128, :])
                else:
                    cols = slice(half * (N // 2), (half + 1) * (N // 2))
                    eng.dma_start(t[:, cols], src[b, cc * 128:(cc + 1) * 128, cols])

            def kt_part(b, nt, w_sb, dstT, which):
                # one 512-col slice of a K^T/Q^T projection: 8 MMs + 1 copy
                ps = psA.tile([128, 2 * MT], fp32, tag="big", name=f"kp{which}{b}{nt}")
                for cc in range(CC):
                    nc.tensor.matmul(
                        ps[:, 0:MT],
                        lhsT=w_sb[:, cc, :],
                        rhs=xch[(which, b, cc)][:, nt * MT:(nt + 1) * MT],
                        start=(cc == 0),
                        stop=(cc == CC - 1),
                    )
                nc.vector.tensor_copy(dstT[:, nt * MT:(nt + 1) * MT], ps[:, 0:MT])

            def v_part(b, q):
                # V rows for key chunks 2q, 2q+1: 16 MMs + 2 copies
                ps = psA.tile([128, 2 * MT], fp32, tag="big", name=f"vp{b}{q}")
                for j in range(2):
                    kc = 2 * q + j
                    for cc in range(CC):
                        nc.tensor.matmul(
                            ps[:, j * 128:(j + 1) * 128],
                            lhsT=xch[("r", b, cc)][:, kc * 128:(kc + 1) * 128],
                            rhs=wv_sb[:, cc, :],
                            start=(cc == 0),
                            stop=(cc == CC - 1),
                        )
                    nc.vector.tensor_copy(
                        vA[b][:, 2 * q + j:2 * q + j + 1, :, 0:HD],
                        ps[:, j * 128:(j + 1) * 128].rearrange(
                            "p (k h d) -> p k h d", k=1, h=HPC
                        ),
                    )

            scale = float(HD) ** -0.5

            def attn_mt(b, mt, fillers=None, av_lag=1, tail=()):
                fillers = fillers or {}
                po = [
                    psO.tile([HD + 1, MT], fp32, tag="o", name=f"po{h}")
                    for h in range(HPC)
                ]

                def av_pair(kc, eS):
                    for h in range(HPC):
                        for j in range(2):
                            nc.tensor.matmul(
                                po[h][:],
                                lhsT=vA[b][:, kc + j, h, :],
                                rhs=eS[h][:, j, :],
                                start=(kc == 0 and j == 0),
                                stop=(kc == KC - 2 and j == 1),
                            )

                # software-pipelined by av_lag kc-pairs: the AV of pair k is
                # emitted after the S^T/exp of pair k+av_lag (and any filler
                # PE work), so the PE always has wait-free work while the ACT
                # engine streams exps back-to-back (ACT is the bottleneck).
                pending = []
                for pi, kc in enumerate(range(0, KC, 2)):
                    pss = [
                        psA.tile([128, 2 * MT], fp32, tag="big", name="pss")
                        for _ in range(HPC)
                    ]
                    for j in range(2):
                        # the two heads sit at partitions 0-63 / 64-127 so the
                        # PE row-groups run their K=64 matmuls concurrently
                        for h in range(HPC):
                            nc.tensor.matmul(
                                pss[h][:, j * MT:(j + 1) * MT],
                                lhsT=kT[b][h * HD:(h + 1) * HD, (kc + j) * 128:(kc + j + 1) * 128],
                                rhs=qT[b][h * HD:(h + 1) * HD, mt * MT:(mt + 1) * MT],
                                start=True,
                                stop=True,
                            )
                    eS = [
                        epool.tile([128, 2, MT], bf16, tag="eS", name="eS")
                        for _ in range(HPC)
                    ]
                    for h in range(HPC):
                        nc.scalar.activation(
                            eS[h][:].rearrange("p a b -> p (a b)"),
                            pss[h][:],
                            AF.Exp,
                            scale=scale,
                        )
                    for f in fillers.get(pi, ()):
                        f()
                    if len(pending) == av_lag:
                        av_pair(*pending.pop(0))
                    pending.append((kc, eS))
                ti = 0
                while pending:
                    if ti < len(tail):
                        tail[ti]()
                        ti += 1
                    av_pair(*pending.pop(0))
                for f in tail[ti:]:
                    f()

                # unnormalized output + denominator rows -> oU / a2a staging.
                # m-tile mt covers dst cores (mt%2)*4+q in collective mt//2.
                hf = mt // 2
                s0 = (mt % 2) * 4
                for h in range(HPC):
                    nc.vector.tensor_copy(
                        oU[b][h * HD:(h + 1) * HD, mt * MT:(mt + 1) * MT],
                        po[h][0:HD, :],
                    )
                    dt = spool.tile([HD + 1, MT], bf16, tag="dt", name="dt")
                    nc.vector.tensor_copy(dt[HD:HD + 1, :], po[h][HD:HD + 1, :])
                    nc.sync.dma_start(
                        a2a_in[b][hf][s0:s0 + 4, CHPC + h:CHPC + h + 1, :],
                        dt[HD:HD + 1, :].rearrange("a (q c) -> a q c", q=4),
                    )
                for q in range(4):
                    nc.sync.dma_start(
                        a2a_in[b][hf][s0 + q][0:CHPC, :],
                        oU[b][:, mt * MT + q * HS:mt * MT + (q + 1) * HS],
                    )

            def fire_a2a(b, hf):
                nc.gpsimd.collective_compute(
                    "AllToAll",
                    mybir.AluOpType.bypass,
                    replica_groups=[list(range(NCORES))],
                    ins=[a2a_in[b][hf][:].opt()],
                    outs=[a2a_out[b][hf][:].opt()],
                )

            def proj(b, hf):
                src = a2a_out[b][hf]
                # dsb rows 0-7 = even heads (h=0 of each src core), 8-15 odd.
                # Allocated from the attention-path "dt" tag ON PURPOSE (see
                # module docstring): the pool WAR (bufs=2) pins this load and
                # the reciprocal chain behind the previous m-tiles' dt casts.
                dsb = spool.tile([HD + 1, MT], bf16, tag="dt", name=f"dsb{b}{hf}")
                for hh in range(2):
                    nc.gpsimd.dma_start(
                        dsb[hh * NCORES:(hh + 1) * NCORES, 0:HS],
                        src[:, CHPC + hh:CHPC + hh + 1, :].rearrange(
                            "i h m -> i (h m)"
                        ),
                    )
                y_sb = ppool.tile([128, NCORES, HS], bf16, tag="y", name=f"y{b}{hf}")
                for i in range(NCORES):
                    nc.gpsimd.dma_start(y_sb[:, i, :], src[i][0:CHPC, :])
                rf = ppool.tile([16, HS], fp32, tag="rf", name=f"rf{b}{hf}")
                nc.vector.reciprocal(rf[:], dsb[0:16, 0:HS])
                rN = ppool.tile([16, HS], bf16, tag="rN", name=f"rN{b}{hf}")
                nc.vector.tensor_copy(rN[:], rf[:])
                nc.gpsimd.dma_start(
                    rdram[b, hf].rearrange("h i m -> (h i) m"), rN[:]
                )
                rb = ppool.tile([128, NCORES, HS], bf16, tag="rb", name=f"rb{b}{hf}")
                for hh in range(2):
                    nc.gpsimd.dma_start(
                        rb[hh * HD:(hh + 1) * HD],
                        rdram[b, hf, hh:hh + 1].to_broadcast((HD, NCORES, HS)),
                    )
                xn = ppool.tile([128, NCORES, HS], bf16, tag="xn", name=f"xn{b}{hf}")
                nc.vector.tensor_mul(xn[:], y_sb[:], rb[:])
                for oc in range(CC):
                    psb = psA.tile([128, 2 * MT], fp32, tag="big", name="pp")
                    ps = psb[:, 0:HS]
                    for cc in range(CC):
                        nc.tensor.matmul(
                            ps[:],
                            lhsT=wp_sb[:, cc, oc * 128:(oc + 1) * 128],
                            rhs=xn[:, cc, :],
                            start=(cc == 0),
                            stop=(cc == CC - 1),
                        )
                    osb = ppool.tile([128, HS], fp32, tag="outsb", name="osb")
                    nc.scalar.activation(
                        osb[:], ps[:], AF.Identity, bias=bias_sb[:, oc:oc + 1]
                    )
                    nc.sync.dma_start(
                        out[b, oc * 128:(oc + 1) * 128, hf * HS:(hf + 1) * HS],
                        osb[:],
                    )

            # ================= emission schedule =================
            # batch-0 loads: the two low halves first, so K-parts 0/1 and
            # Q-part 0 (the minimum before attention starts) land earliest
            E3 = [nc.sync, nc.gpsimd, nc.scalar]
            for cc in range(CC):
                load_chunk("r", 0, cc, half=0, eng=E3[cc % 3])
            for cc in range(CC):
                load_chunk("t", 0, cc, half=0, eng=E3[cc % 3])
            for cc in range(CC):
                load_chunk("r", 0, cc, half=1, eng=E3[cc % 3])
            for cc in range(CC):
                load_chunk("t", 0, cc, half=1, eng=E3[cc % 3])
            # wproj/bias after the batch-0 chunks on the scalar queue; needed
            # only by proj() mid-kernel
            wp_sb = wpool.tile([128, CC, C], bf16, name="wp_sb")
            for cc in range(CC):
                nc.scalar.dma_start(wp_sb[:, cc, :], wproj[cc * 128:(cc + 1) * 128, :])
            bias_sb = wpool.tile([128, CC], fp32, name="bias_sb")
            nc.scalar.dma_start(bias_sb[:], bproj.ap().rearrange("(a p) -> p a", p=128))

            # the minimum PE work before attention m-tile 0 (pairs 0-3 only
            # need K^T keys 0-1023 and Q^T cols 0-511)
            kt_part(0, 0, wk_sb, kT[0], "r")
            kt_part(0, 1, wk_sb, kT[0], "r")
            kt_part(0, 0, wq_sb, qT[0], "t")

            # everything else rides in attention-pair filler slots; av_lag=2
            # gives the just-in-time V parts one pair of slack
            P = functools.partial
            attn_mt(
                0, 0,
                fillers={
                    0: (P(kt_part, 0, 2, wk_sb, kT[0], "r"),),
                    1: (P(kt_part, 0, 3, wk_sb, kT[0], "r"),),
                    **{pi: (P(v_part, 0, pi - 2),) for pi in range(2, 8)},
                },
                av_lag=2,
                tail=(
                    P(v_part, 0, 6),
                    P(v_part, 0, 7),
                    P(kt_part, 0, 1, wq_sb, qT[0], "t"),
                ),
            )
            # batch-1 chunk loads: emitted only now so their pool-recycling
            # waits (on r0/t0 release) sit behind this m-tile's staging in
            # the sync/gpsimd queues (scalar stays exp-only)
            E2 = [nc.sync, nc.gpsimd]
            for cc in range(CC):
                load_chunk("r", 1, cc, eng=E2[cc % 2])
            attn_mt(0, 1, {
                0: (P(kt_part, 0, 2, wq_sb, qT[0], "t"),),
                2: (P(kt_part, 0, 3, wq_sb, qT[0], "t"),),
            })
            fire_a2a(0, 0)
            for cc in range(CC):
                load_chunk("t", 1, cc, eng=E2[cc % 2])
            attn_mt(0, 2, {
                pi: (P(kt_part, 1, pi, wk_sb, kT[1], "r"),)
                for pi in range(4)
            })
            attn_mt(0, 3, {
                0: (P(kt_part, 1, 0, wq_sb, qT[1], "t"),),
                2: (P(kt_part, 1, 1, wq_sb, qT[1], "t"),),
            })
            fire_a2a(0, 1)
            attn_mt(1, 0, {pi: (P(v_part, 1, pi),) for pi in range(8)})
            attn_mt(1, 1, {
                0: (P(kt_part, 1, 2, wq_sb, qT[1], "t"),),
                2: (P(kt_part, 1, 3, wq_sb, qT[1], "t"),),
            })
            fire_a2a(1, 0)
            attn_mt(1, 2)
            attn_mt(1, 3)
            fire_a2a(1, 1)
            # proj halves run strictly post-attention (dt-pool WAR) and
            # pipeline with the remaining collectives, keeping the PE warm
            proj(0, 0)
            proj(0, 1)
            proj(1, 0)
            proj(1, 1)

    nc.compile()
    return nc


def _shard_inputs(reference_data, target_data, Wq, Wkv, Wproj, bproj):
    import ml_dtypes

    bf16 = ml_dtypes.bfloat16
    xrefT = np.ascontiguousarray(
        np.asarray(reference_data, dtype=np.float32).transpose(0, 2, 1)
    ).astype(bf16)
    xtgtT = np.ascontiguousarray(
        np.asarray(target_data, dtype=np.float32).transpose(0, 2, 1)
    ).astype(bf16)
    Wq = np.asarray(Wq, dtype=np.float32)
    Wkv = np.asarray(Wkv, dtype=np.float32)
    Wproj_b = np.asarray(Wproj, dtype=np.float32).astype(bf16)
    bproj = np.asarray(bproj, dtype=np.float32)

    in_maps = []
    for c in range(NCORES):
        lo, hi = c * CHPC, (c + 1) * CHPC
        in_maps.append(
            {
                "xrefT": xrefT,
                "xtgtT": xtgtT,
                "wq": Wq[:, lo:hi].astype(bf16),
                "wk": Wkv[:, lo:hi].astype(bf16),
                "wv": Wkv[:, C + lo:C + hi].astype(bf16),
                "wproj": Wproj_b,
                "bproj": bproj,
            }
        )
    return in_maps


def _ensure_ntff_hook():
    """Register the axon NTFF profile hook if the image's antenv lacks it."""
    try:
        import antenv.axon_hooks  # noqa: F401

        return
    except ImportError:
        pass
    import sys
    import types

    import antenv

    mod = types.ModuleType("antenv.axon_hooks")
    state = {"hook": None}
    mod.set_axon_ntff_profile_hook = lambda h: state.__setitem__("hook", h)
    mod.get_axon_ntff_profile_hook = lambda: state["hook"]
    sys.modules["antenv.axon_hooks"] = mod
    antenv.axon_hooks = mod
    try:
        from trn_agent_boot.trn_boot import _ntff_profile_via_ctypes

        mod.set_axon_ntff_profile_hook(
            _ntff_profile_via_ctypes("/opt/axon/libaxon_pjrt.so")
        )
    except Exception:
        pass


def run(inputs: dict, trace: bool = False):
    """Compile (cached), run on 8 cores, return (full_output, BassKernelResults)."""
    from concourse.bass_utils import run_bass_kernel_spmd

    if trace:
        _ensure_ntff_hook()
    nc = _build()
    in_maps = _shard_inputs(**inputs)
    res = run_bass_kernel_spmd(
        nc, in_maps, core_ids=list(range(NCORES)), trace=trace
    )
    return _assemble(res), res


def _assemble(res):
    full = np.zeros((B, M, C), dtype=np.float32)
    hs = MBLK // 2
    for c in range(NCORES):
        blk = np.asarray(res.results[c]["out"], dtype=np.float32)  # [B, C, MBLK]
        for b in range(B):
            for hf in range(2):
                full[b, 1024 * hf + c * hs:1024 * hf + (c + 1) * hs, :] = (
                    blk[b][:, hf * hs:(hf + 1) * hs].T
                )
    return full


def kernel(reference_data, target_data, Wq, Wkv, Wproj, bproj) -> np.ndarray:
    full, _ = run(
        {
            "reference_data": reference_data,
            "target_data": target_data,
            "Wq": Wq,
            "Wkv": Wkv,
            "Wproj": Wproj,
            "bproj": bproj,
        }
    )
    return full
